# revision 23
# baseline (speedup 1.0000x reference)
"""Trainium2 Bass kernel: contrastive KL loss (nn_Contrastive_loss).

Reference computation (per batch b):
    tic_n  = tic / max_t(tic)
    d[p,t] = tic_n[p] - tic_n[t];  pos = exp(-d^2/2);  tw = pos / rowsum(pos)
    ne     = embedding / ||embedding||;  sc = ne@ne.T + 1;  iw = sc / rowsum(sc)
    loss   = mean_{b,p} sum_t tw * (log tw - log(iw + 1e-6))

Restructured for the hardware (validated to ~5e-8 vs reference):
    rowsum(sc) = (ne_p . sum_t ne_t) + T            (matvec, no elementwise pass)
    ln_in      = Ln(G + 1 + eps*sc_rs)              (ACT, direct from PSUM)
    d'         = (tic_t - tic_p) * rmx/sqrt(2)      (DVE tensor_scalar, fp32 2x mode)
    d2'        = Square(d');  pos = Exp(-d2') with accum_out -> pos_rs
    u          = d2' + ln_in;  pu = sum_t pos*u     (DVE tensor_tensor_reduce)
    row_kl(p)  = ln(sc_rs) - ln(pos_rs) - pu/pos_rs
    loss       = sum(row_kl) / (B*T)

Sharding: data-parallel over B across 8 cores (4 batches each). Each core
writes per-partition partial sums [128,1]; the host sums and scales.
"""

import os
import numpy as np

os.environ.setdefault("MYCRO_LOCAL_CACHE", "1")

B, T, D = 32, 2048, 64
N_CORES = 8
B_LOC = B // N_CORES      # batches per core
NBLK = T // 128           # anchor blocks per batch
CH = 512                  # PSUM matmul chunk width
NCH = T // CH
NCOL = B_LOC * NBLK       # block columns per core
LOG_EPS = 1e-6
INV_SQRT2 = 0.7071067811865476
# pos is computed as Derivative_Erf(d') = (2/sqrt(pi))*exp(-d'^2); the scale
# cancels in pu/pos_rs and shifts ln(pos_rs) by -ln(2/sqrt(pi)) per row.
LN_2_OVER_SQRT_PI = 0.12078223763524522

_PROGRAM = None


def _build_program():
    from contextlib import ExitStack
    import concourse.bass as bass
    import concourse.tile as tile
    from concourse import bacc, mybir, masks

    f32 = mybir.dt.float32
    AF = mybir.ActivationFunctionType
    OP = mybir.AluOpType
    AX = mybir.AxisListType

    nc = bacc.Bacc(
        "TRN2", target_bir_lowering=False, debug=False, num_devices=N_CORES
    )
    emb_d = nc.dram_tensor("embedding", [B_LOC, T, D], f32, kind="ExternalInput").ap()
    tic_d = nc.dram_tensor("tic", [B_LOC, T], f32, kind="ExternalInput").ap()
    out_d = nc.dram_tensor("out", [128, 1], f32, kind="ExternalOutput").ap()

    with tile.TileContext(nc) as tc, ExitStack() as ctx:
        consts = ctx.enter_context(tc.tile_pool(name="consts", bufs=1))
        accp = ctx.enter_context(tc.tile_pool(name="acc", bufs=1))
        bpre = ctx.enter_context(tc.tile_pool(name="bpre", bufs=2))
        epre = ctx.enter_context(tc.tile_pool(name="epre", bufs=3))
        big = ctx.enter_context(tc.tile_pool(name="big", bufs=6))
        big2 = ctx.enter_context(tc.tile_pool(name="big2", bufs=3))
        small = ctx.enter_context(tc.tile_pool(name="small", bufs=6))
        psum = ctx.enter_context(
            tc.tile_pool(name="psum", bufs=2, space=bass.MemorySpace.PSUM)
        )
        psum_tr = ctx.enter_context(
            tc.tile_pool(name="psum_tr", bufs=2, space=bass.MemorySpace.PSUM)
        )
        psum_rs = ctx.enter_context(
            tc.tile_pool(name="psum_rs", bufs=1, space=bass.MemorySpace.PSUM)
        )
        psum_sv = ctx.enter_context(
            tc.tile_pool(name="psum_sv", bufs=1, space=bass.MemorySpace.PSUM)
        )

        bf16 = mybir.dt.bfloat16
        identity = consts.tile([128, 128], bf16)
        masks.make_identity(nc, identity[:])
        ones_row = consts.tile([1, 128], f32)
        nc.gpsimd.memset(ones_row[:], 1.0)
        ones_col = consts.tile([128, 1], bf16)
        nc.gpsimd.memset(ones_col[:], 1.0)
        bias_T = consts.tile([128, 1], f32)
        nc.gpsimd.memset(bias_T[:], float(T))

        pu_all = accp.tile([128, NCOL], f32)
        prs_all = accp.tile([128, NCOL], f32)
        srs_all = accp.tile([128, NCOL], f32)

        for b in range(B_LOC):
            # ---- tic preprocessing ----
            ticrow = bpre.tile([1, T], f32, tag="ticrow")
            nc.sync.dma_start(ticrow[:], tic_d[b : b + 1, :])
            mx = bpre.tile([1, 1], f32, tag="mx")
            nc.vector.reduce_max(mx[:], ticrow[:], axis=AX.X)
            rmx = bpre.tile([1, 1], f32, tag="rmx")
            nc.vector.reciprocal(rmx[:], mx[:])
            # broadcast rmx across partitions, scaled by 1/sqrt(2)
            rmx_ps = psum_rs.tile([128, 1], f32, tag="rs")
            nc.tensor.matmul(rmx_ps[:], ones_row[:], rmx[:])
            rmxh = bpre.tile([128, 1], f32, tag="rmxh")
            nc.scalar.activation(rmxh[:], rmx_ps[:], AF.Copy, bias=0.0, scale=INV_SQRT2)
            # raw tic broadcast across partitions: tbc[p, t] = tic[b, t]
            tbc = bpre.tile([128, T], bf16, tag="tbc")
            for c in range(NCH):
                tb_ps = psum_tr.tile([128, CH], f32, tag="x")
                nc.tensor.matmul(tb_ps[:], ones_row[:], ticrow[:, c * CH : (c + 1) * CH])
                nc.scalar.copy(tbc[:, c * CH : (c + 1) * CH], tb_ps[:])
            # per-anchor tic columns: tpr[p, k] = tic[b, 128k + p]
            tpr = bpre.tile([128, NBLK], f32, tag="tpr")
            nc.sync.dma_start(tpr[:], tic_d[b].rearrange("(g p) -> p g", p=128))

            # ---- embedding preprocessing ----
            # Phase 1: load all 16 tiles, batched Square+accum -> squared norms.
            neT = bpre.tile([64, T], bf16, tag="neT")
            svec_ps = psum_sv.tile([64, 1], f32, tag="sv")
            n2_all = bpre.tile([128, NBLK], f32, tag="n2all")
            ets = []
            for k in range(NBLK):
                et = epre.tile([128, D], f32, tag=f"et{k}")
                nc.sync.dma_start(et[:], emb_d[b, k * 128 : (k + 1) * 128, :])
                ets.append(et)
                sq = epre.tile([128, D], f32, tag="sq")
                nc.scalar.activation(sq[:], et[:], AF.Square, accum_out=n2_all[:, k : k + 1])
            # Phase 2: one Sqrt + one reciprocal for all 16 norms.
            nrm_all = bpre.tile([128, NBLK], f32, tag="nrmall")
            nc.scalar.activation(nrm_all[:], n2_all[:], AF.Sqrt)
            rinv_all = bpre.tile([128, NBLK], f32, tag="rinvall")
            nc.vector.reciprocal(rinv_all[:], nrm_all[:])
            # Phase 3: normalize (cast bf16), svec accumulation, transpose.
            for k in range(NBLK):
                ne = epre.tile([128, D], bf16, tag="ne")
                nc.vector.tensor_scalar(
                    out=ne[:], in0=ets[k][:], scalar1=rinv_all[:, k : k + 1],
                    scalar2=None, op0=OP.mult,
                )
                # svec += ne.T @ ones  (accumulated in PSUM across the 16 tiles)
                nc.tensor.matmul(
                    svec_ps[:], ne[:], ones_col[:],
                    start=(k == 0), stop=(k == NBLK - 1),
                )
                t_ps = psum_tr.tile([64, 128], bf16, tag="x")
                nc.tensor.transpose(t_ps[:], ne[:], identity[:])
                nc.scalar.copy(neT[:, k * 128 : (k + 1) * 128], t_ps[:])
            svec = bpre.tile([64, 1], bf16, tag="svecsb")
            nc.vector.tensor_copy(svec[:], svec_ps[:])

            # ---- anchor-block loop, G=4 batching to cut ACT table reloads ----
            G4 = 4
            for k0 in range(0, NBLK, G4):
                ks = list(range(k0, min(k0 + G4, NBLK)))
                bias_lns, dps, d2s, poss, lnfs = {}, {}, {}, {}, {}
                for k in ks:
                    col = b * NBLK + k
                    pT = neT[:, k * 128 : (k + 1) * 128]
                    rs_ps = psum_rs.tile([128, 1], f32, tag="rs")
                    nc.tensor.matmul(rs_ps[:], pT, svec[:])
                    nc.vector.tensor_copy(srs_all[:, col : col + 1], rs_ps[:])
                    bias_ln = small.tile([128, 1], f32, tag="bln")
                    nc.vector.tensor_scalar(
                        out=bias_ln[:], in0=rs_ps[:], scalar1=LOG_EPS,
                        scalar2=1.0 + T * LOG_EPS, op0=OP.mult, op1=OP.add,
                    )
                    bias_lns[k] = bias_ln
                    dp = big.tile([128, T], bf16, tag="dp")
                    nc.vector.tensor_scalar(
                        out=dp[:], in0=tbc[:], scalar1=tpr[:, k : k + 1],
                        scalar2=rmxh[:], op0=OP.subtract, op1=OP.mult,
                    )
                    dps[k] = dp
                # pos = (2/sqrt(pi))*exp(-dp^2) in ONE LUT op; the constant
                # factor cancels in pu/pos_rs and is folded into the host loss.
                for k in ks:
                    col = b * NBLK + k
                    pos = big.tile([128, T], bf16, tag="pos")
                    nc.scalar.activation(
                        pos[:], dps[k][:], AF.Derivative_Erf,
                        accum_out=prs_all[:, col : col + 1],
                    )
                    poss[k] = pos
                # d2 = dp^2: first group per batch on ScalarE (batched -> one
                # LUT load), the rest on DVE (bf16 2x) to balance the engines.
                for k in ks:
                    d2 = big.tile([128, T], bf16, tag="d2")
                    if k0 == 0:
                        nc.scalar.activation(d2[:], dps[k][:], AF.Square)
                    else:
                        nc.vector.tensor_mul(d2[:], dps[k][:], dps[k][:])
                    d2s[k] = d2
                for k in ks:
                    pT = neT[:, k * 128 : (k + 1) * 128]
                    lnf = big.tile([128, T], bf16, tag="lnf")
                    for c in range(2):
                        g_ps = psum.tile([128, 1024], f32, tag="g")
                        nc.tensor.matmul(
                            g_ps[:, 0:CH], pT, neT[:, (2 * c) * CH : (2 * c + 1) * CH]
                        )
                        nc.tensor.matmul(
                            g_ps[:, CH:1024], pT,
                            neT[:, (2 * c + 1) * CH : (2 * c + 2) * CH],
                        )
                        nc.scalar.activation(
                            lnf[:, c * 1024 : (c + 1) * 1024], g_ps[:], AF.Ln,
                            bias=bias_lns[k][:], scale=1.0,
                        )
                    lnfs[k] = lnf
                for k in ks:
                    col = b * NBLK + k
                    u = big2.tile([128, T], bf16, tag="u")
                    nc.vector.tensor_add(u[:], d2s[k][:], lnfs[k][:])
                    scr = big2.tile([128, T], bf16, tag="scr")
                    nc.vector.tensor_mul(scr[:], poss[k][:], u[:])
                    # fused reduce: tensor_scalar bypass-mul with accum (4x mode)
                    scr2 = big2.tile([128, T], bf16, tag="scr2")
                    nc.vector.tensor_scalar(
                        out=scr2[:], in0=scr[:], scalar1=1.0, scalar2=None,
                        op0=OP.mult, op1=OP.add,
                        accum_out=pu_all[:, col : col + 1],
                    )

        # ---- finals: row_kl = ln(srs+T) - ln(prs) - pu/prs, summed ----
        lnp = accp.tile([128, NCOL], f32)
        nc.scalar.activation(lnp[:], prs_all[:], AF.Ln)
        lns = accp.tile([128, NCOL], f32)
        nc.scalar.activation(lns[:], srs_all[:], AF.Ln, bias=bias_T[:])
        rpr = accp.tile([128, NCOL], f32)
        nc.vector.reciprocal(rpr[:], prs_all[:])
        cc = accp.tile([128, NCOL], f32)
        nc.vector.tensor_sub(cc[:], lns[:], lnp[:])
        pq = accp.tile([128, NCOL], f32)
        nc.vector.tensor_mul(pq[:], pu_all[:], rpr[:])
        rows = accp.tile([128, NCOL], f32)
        nc.vector.tensor_sub(rows[:], cc[:], pq[:])
        tot = accp.tile([128, 1], f32)
        nc.vector.reduce_sum(tot[:], rows[:], axis=AX.X)
        nc.sync.dma_start(out_d[:], tot[:])

    nc.compile()
    return nc


def _get_program():
    global _PROGRAM
    if _PROGRAM is None:
        _PROGRAM = _build_program()
    return _PROGRAM


def _install_ntff_hook():
    """Register the axon NTFF profiling hook that the boot script skips when
    antenv.axon_hooks is absent (test/profiling path only)."""
    import sys
    import types

    if "antenv.axon_hooks" not in sys.modules:
        import antenv

        mod = types.ModuleType("antenv.axon_hooks")
        state = {"hook": None}
        mod.set_axon_ntff_profile_hook = lambda h: state.__setitem__("hook", h)
        mod.get_axon_ntff_profile_hook = lambda: state["hook"]
        sys.modules["antenv.axon_hooks"] = mod
        antenv.axon_hooks = mod
        from trn_agent_boot.trn_boot import _ntff_profile_via_ctypes

        mod.set_axon_ntff_profile_hook(
            _ntff_profile_via_ctypes("/opt/axon/libaxon_pjrt.so")
        )
    from concourse import bass_utils

    bass_utils.upload_artifacts = lambda tmpdir: tmpdir


def kernel(embedding, tic, _trace=False):
    embedding = np.ascontiguousarray(embedding, dtype=np.float32)
    tic = np.ascontiguousarray(tic, dtype=np.float32)
    assert embedding.shape == (B, T, D) and tic.shape == (B, T)

    from concourse.bass_utils import run_bass_kernel_spmd

    if _trace:
        _install_ntff_hook()
    try:
        import ctypes
        import jax
        jax.devices()
        ctypes.CDLL("/opt/axon/libaxon_pjrt.so").axon_reset()
    except Exception:
        pass
    nc = _get_program()
    in_maps = [
        {
            "embedding": embedding[i * B_LOC : (i + 1) * B_LOC],
            "tic": tic[i * B_LOC : (i + 1) * B_LOC],
        }
        for i in range(N_CORES)
    ]
    res = run_bass_kernel_spmd(nc, in_maps, list(range(N_CORES)), trace=_trace)
    total = sum(float(r["out"].sum()) for r in res.results)
    loss = np.array(total / (B * T) + LN_2_OVER_SQRT_PI, dtype=np.float32)
    if _trace:
        return loss, res
    return loss


# revision 25
# speedup vs baseline: 1.1151x; 1.1151x over previous
"""Trainium2 Bass kernel: contrastive KL loss (nn_Contrastive_loss).

Reference computation (per batch b):
    tic_n  = tic / max_t(tic)
    d[p,t] = tic_n[p] - tic_n[t];  pos = exp(-d^2/2);  tw = pos / rowsum(pos)
    ne     = embedding / ||embedding||;  sc = ne@ne.T + 1;  iw = sc / rowsum(sc)
    loss   = mean_{b,p} sum_t tw * (log tw - log(iw + 1e-6))

Restructured for the hardware (validated to ~5e-8 vs reference):
    rowsum(sc) = (ne_p . sum_t ne_t) + T            (matvec, no elementwise pass)
    ln_in      = Ln(G + 1 + eps*sc_rs)              (ACT, direct from PSUM)
    d'         = (tic_t - tic_p) * rmx/sqrt(2)      (DVE tensor_scalar, fp32 2x mode)
    d2'        = Square(d');  pos = Exp(-d2') with accum_out -> pos_rs
    u          = d2' + ln_in;  pu = sum_t pos*u     (DVE tensor_tensor_reduce)
    row_kl(p)  = ln(sc_rs) - ln(pos_rs) - pu/pos_rs
    loss       = sum(row_kl) / (B*T)

Sharding: data-parallel over B across 8 cores (4 batches each). Each core
writes per-partition partial sums [128,1]; the host sums and scales.
"""

import os
import numpy as np

os.environ.setdefault("MYCRO_LOCAL_CACHE", "1")

B, T, D = 32, 2048, 64
N_CORES = 8
B_LOC = B // N_CORES      # batches per core
NBLK = T // 128           # anchor blocks per batch
CH = 512                  # PSUM matmul chunk width
NCH = T // CH
NCOL = B_LOC * NBLK       # block columns per core
LOG_EPS = 1e-6
INV_SQRT2 = 0.7071067811865476
# pos is computed as Derivative_Erf(d') = (2/sqrt(pi))*exp(-d'^2); the scale
# cancels in pu/pos_rs and shifts ln(pos_rs) by -ln(2/sqrt(pi)) per row.
LN_2_OVER_SQRT_PI = 0.12078223763524522

_PROGRAM = None


def _build_program():
    from contextlib import ExitStack
    import concourse.bass as bass
    import concourse.tile as tile
    from concourse import bacc, mybir, masks

    f32 = mybir.dt.float32
    AF = mybir.ActivationFunctionType
    OP = mybir.AluOpType
    AX = mybir.AxisListType

    nc = bacc.Bacc(
        "TRN2", target_bir_lowering=False, debug=False, num_devices=N_CORES
    )
    emb_d = nc.dram_tensor("embedding", [B_LOC, T, D], f32, kind="ExternalInput").ap()
    tic_d = nc.dram_tensor("tic", [B_LOC, T], f32, kind="ExternalInput").ap()
    out_d = nc.dram_tensor("out", [128, 1], f32, kind="ExternalOutput").ap()

    with tile.TileContext(nc) as tc, ExitStack() as ctx:
        consts = ctx.enter_context(tc.tile_pool(name="consts", bufs=1))
        accp = ctx.enter_context(tc.tile_pool(name="acc", bufs=1))
        bpre = ctx.enter_context(tc.tile_pool(name="bpre", bufs=2))
        epre = ctx.enter_context(tc.tile_pool(name="epre", bufs=3))
        big = ctx.enter_context(tc.tile_pool(name="big", bufs=6))
        big2 = ctx.enter_context(tc.tile_pool(name="big2", bufs=3))
        small = ctx.enter_context(tc.tile_pool(name="small", bufs=6))
        psum = ctx.enter_context(
            tc.tile_pool(name="psum", bufs=2, space=bass.MemorySpace.PSUM)
        )
        psum_tr = ctx.enter_context(
            tc.tile_pool(name="psum_tr", bufs=2, space=bass.MemorySpace.PSUM)
        )
        psum_rs = ctx.enter_context(
            tc.tile_pool(name="psum_rs", bufs=1, space=bass.MemorySpace.PSUM)
        )
        psum_sv = ctx.enter_context(
            tc.tile_pool(name="psum_sv", bufs=1, space=bass.MemorySpace.PSUM)
        )

        bf16 = mybir.dt.bfloat16
        identity = consts.tile([128, 128], bf16)
        masks.make_identity(nc, identity[:])
        ones_row = consts.tile([1, 128], f32)
        nc.gpsimd.memset(ones_row[:], 1.0)
        ones_col = consts.tile([128, 1], bf16)
        nc.gpsimd.memset(ones_col[:], 1.0)
        bias_T = consts.tile([128, 1], f32)
        nc.gpsimd.memset(bias_T[:], float(T))

        pu_all = accp.tile([128, NCOL], f32)
        prs_all = accp.tile([128, NCOL], f32)
        srs_all = accp.tile([128, NCOL], f32)

        for b in range(B_LOC):
            # ---- tic preprocessing ----
            ticrow = bpre.tile([1, T], f32, tag="ticrow")
            nc.sync.dma_start(ticrow[:], tic_d[b : b + 1, :])
            mx = bpre.tile([1, 1], f32, tag="mx")
            nc.vector.reduce_max(mx[:], ticrow[:], axis=AX.X)
            rmx = bpre.tile([1, 1], f32, tag="rmx")
            nc.vector.reciprocal(rmx[:], mx[:])
            # broadcast rmx across partitions, scaled by 1/sqrt(2)
            rmx_ps = psum_rs.tile([128, 1], f32, tag="rs")
            nc.tensor.matmul(rmx_ps[:], ones_row[:], rmx[:])
            rmxh = bpre.tile([128, 1], f32, tag="rmxh")
            nc.scalar.activation(rmxh[:], rmx_ps[:], AF.Copy, bias=0.0, scale=INV_SQRT2)
            # raw tic broadcast across partitions: tbc[p, t] = tic[b, t]
            tbc = bpre.tile([128, T], bf16, tag="tbc")
            for c in range(NCH):
                tb_ps = psum_tr.tile([128, CH], f32, tag="x")
                nc.tensor.matmul(tb_ps[:], ones_row[:], ticrow[:, c * CH : (c + 1) * CH])
                nc.scalar.copy(tbc[:, c * CH : (c + 1) * CH], tb_ps[:])
            # per-anchor tic columns: tpr[p, k] = tic[b, 128k + p]
            tpr = bpre.tile([128, NBLK], f32, tag="tpr")
            nc.sync.dma_start(tpr[:], tic_d[b].rearrange("(g p) -> p g", p=128))

            # ---- embedding preprocessing ----
            # Phase 1: load all 16 tiles, batched Square+accum -> squared norms.
            neT = bpre.tile([64, T], bf16, tag="neT")
            svec_ps = psum_sv.tile([64, 1], f32, tag="sv")
            n2_all = bpre.tile([128, NBLK], f32, tag="n2all")
            ets = []
            for k in range(NBLK):
                et = epre.tile([128, D], f32, tag=f"et{k}")
                nc.sync.dma_start(et[:], emb_d[b, k * 128 : (k + 1) * 128, :])
                ets.append(et)
                sq = epre.tile([128, D], f32, tag="sq")
                nc.scalar.activation(sq[:], et[:], AF.Square, accum_out=n2_all[:, k : k + 1])
            # Phase 2: one Sqrt + one reciprocal for all 16 norms.
            nrm_all = bpre.tile([128, NBLK], f32, tag="nrmall")
            nc.scalar.activation(nrm_all[:], n2_all[:], AF.Sqrt)
            rinv_all = bpre.tile([128, NBLK], f32, tag="rinvall")
            nc.vector.reciprocal(rinv_all[:], nrm_all[:])
            # Phase 3: normalize (cast bf16), svec accumulation, transpose.
            for k in range(NBLK):
                ne = epre.tile([128, D], bf16, tag="ne")
                nc.vector.tensor_scalar(
                    out=ne[:], in0=ets[k][:], scalar1=rinv_all[:, k : k + 1],
                    scalar2=None, op0=OP.mult,
                )
                # svec += ne.T @ ones  (accumulated in PSUM across the 16 tiles)
                nc.tensor.matmul(
                    svec_ps[:], ne[:], ones_col[:],
                    start=(k == 0), stop=(k == NBLK - 1),
                )
                t_ps = psum_tr.tile([64, 128], bf16, tag="x")
                nc.tensor.transpose(t_ps[:], ne[:], identity[:])
                if k % 2 == 0:
                    nc.vector.tensor_copy(neT[:, k * 128 : (k + 1) * 128], t_ps[:])
                else:
                    nc.scalar.copy(neT[:, k * 128 : (k + 1) * 128], t_ps[:])
            svec = bpre.tile([64, 1], bf16, tag="svecsb")
            nc.vector.tensor_copy(svec[:], svec_ps[:])

            # ---- anchor-block loop, G=4 batching to cut ACT table reloads ----
            G4 = 4
            for k0 in range(0, NBLK, G4):
                ks = list(range(k0, min(k0 + G4, NBLK)))
                bias_lns, dps, d2s, poss, lnfs = {}, {}, {}, {}, {}
                for k in ks:
                    col = b * NBLK + k
                    pT = neT[:, k * 128 : (k + 1) * 128]
                    rs_ps = psum_rs.tile([128, 1], f32, tag="rs")
                    nc.tensor.matmul(rs_ps[:], pT, svec[:])
                    nc.vector.tensor_copy(srs_all[:, col : col + 1], rs_ps[:])
                    bias_ln = small.tile([128, 1], f32, tag="bln")
                    nc.vector.tensor_scalar(
                        out=bias_ln[:], in0=rs_ps[:], scalar1=LOG_EPS,
                        scalar2=1.0 + T * LOG_EPS, op0=OP.mult, op1=OP.add,
                    )
                    bias_lns[k] = bias_ln
                    dp = big.tile([128, T], bf16, tag="dp")
                    nc.vector.tensor_scalar(
                        out=dp[:], in0=tbc[:], scalar1=tpr[:, k : k + 1],
                        scalar2=rmxh[:], op0=OP.subtract, op1=OP.mult,
                    )
                    dps[k] = dp
                # pos = (2/sqrt(pi))*exp(-dp^2) in ONE LUT op; the constant
                # factor cancels in pu/pos_rs and is folded into the host loss.
                for k in ks:
                    col = b * NBLK + k
                    pos = big.tile([128, T], bf16, tag="pos")
                    nc.scalar.activation(
                        pos[:], dps[k][:], AF.Derivative_Erf,
                        accum_out=prs_all[:, col : col + 1],
                    )
                    poss[k] = pos
                # d2 = dp^2 on DVE (bf16 2x); keeping ScalarE to DErf+Ln only
                # minimizes ACT LUT reloads (the scheduler reorders ACT ops,
                # so only a uniform function mix stays cheap).
                for k in ks:
                    d2 = big.tile([128, T], bf16, tag="d2")
                    nc.vector.tensor_mul(d2[:], dps[k][:], dps[k][:])
                    d2s[k] = d2
                for k in ks:
                    pT = neT[:, k * 128 : (k + 1) * 128]
                    lnf = big.tile([128, T], bf16, tag="lnf")
                    for c in range(2):
                        g_ps = psum.tile([128, 1024], f32, tag="g")
                        nc.tensor.matmul(
                            g_ps[:, 0:CH], pT, neT[:, (2 * c) * CH : (2 * c + 1) * CH]
                        )
                        nc.tensor.matmul(
                            g_ps[:, CH:1024], pT,
                            neT[:, (2 * c + 1) * CH : (2 * c + 2) * CH],
                        )
                        nc.scalar.activation(
                            lnf[:, c * 1024 : (c + 1) * 1024], g_ps[:], AF.Ln,
                            bias=bias_lns[k][:], scale=1.0,
                        )
                    lnfs[k] = lnf
                for k in ks:
                    col = b * NBLK + k
                    u = big2.tile([128, T], bf16, tag="u")
                    nc.vector.tensor_add(u[:], d2s[k][:], lnfs[k][:])
                    scr = big2.tile([128, T], bf16, tag="scr")
                    nc.vector.tensor_mul(scr[:], poss[k][:], u[:])
                    # fused reduce: tensor_scalar bypass-mul with accum (4x mode)
                    scr2 = big2.tile([128, T], bf16, tag="scr2")
                    nc.vector.tensor_scalar(
                        out=scr2[:], in0=scr[:], scalar1=1.0, scalar2=None,
                        op0=OP.mult, op1=OP.add,
                        accum_out=pu_all[:, col : col + 1],
                    )

        # ---- finals: row_kl = ln(srs+T) - ln(prs) - pu/prs, summed ----
        lnp = accp.tile([128, NCOL], f32)
        nc.scalar.activation(lnp[:], prs_all[:], AF.Ln)
        lns = accp.tile([128, NCOL], f32)
        nc.scalar.activation(lns[:], srs_all[:], AF.Ln, bias=bias_T[:])
        rpr = accp.tile([128, NCOL], f32)
        nc.vector.reciprocal(rpr[:], prs_all[:])
        cc = accp.tile([128, NCOL], f32)
        nc.vector.tensor_sub(cc[:], lns[:], lnp[:])
        pq = accp.tile([128, NCOL], f32)
        nc.vector.tensor_mul(pq[:], pu_all[:], rpr[:])
        rows = accp.tile([128, NCOL], f32)
        nc.vector.tensor_sub(rows[:], cc[:], pq[:])
        tot = accp.tile([128, 1], f32)
        nc.vector.reduce_sum(tot[:], rows[:], axis=AX.X)
        nc.sync.dma_start(out_d[:], tot[:])

    nc.compile()
    return nc


def _get_program():
    global _PROGRAM
    if _PROGRAM is None:
        _PROGRAM = _build_program()
    return _PROGRAM


def _install_ntff_hook():
    """Register the axon NTFF profiling hook that the boot script skips when
    antenv.axon_hooks is absent (test/profiling path only)."""
    import sys
    import types

    if "antenv.axon_hooks" not in sys.modules:
        import antenv

        mod = types.ModuleType("antenv.axon_hooks")
        state = {"hook": None}
        mod.set_axon_ntff_profile_hook = lambda h: state.__setitem__("hook", h)
        mod.get_axon_ntff_profile_hook = lambda: state["hook"]
        sys.modules["antenv.axon_hooks"] = mod
        antenv.axon_hooks = mod
        from trn_agent_boot.trn_boot import _ntff_profile_via_ctypes

        mod.set_axon_ntff_profile_hook(
            _ntff_profile_via_ctypes("/opt/axon/libaxon_pjrt.so")
        )
    from concourse import bass_utils

    bass_utils.upload_artifacts = lambda tmpdir: tmpdir


def kernel(embedding, tic, _trace=False):
    embedding = np.ascontiguousarray(embedding, dtype=np.float32)
    tic = np.ascontiguousarray(tic, dtype=np.float32)
    assert embedding.shape == (B, T, D) and tic.shape == (B, T)

    from concourse.bass_utils import run_bass_kernel_spmd

    if _trace:
        _install_ntff_hook()
    try:
        import ctypes
        import jax
        jax.devices()
        ctypes.CDLL("/opt/axon/libaxon_pjrt.so").axon_reset()
    except Exception:
        pass
    nc = _get_program()
    in_maps = [
        {
            "embedding": embedding[i * B_LOC : (i + 1) * B_LOC],
            "tic": tic[i * B_LOC : (i + 1) * B_LOC],
        }
        for i in range(N_CORES)
    ]
    res = run_bass_kernel_spmd(nc, in_maps, list(range(N_CORES)), trace=_trace)
    total = sum(float(r["out"].sum()) for r in res.results)
    loss = np.array(total / (B * T) + LN_2_OVER_SQRT_PI, dtype=np.float32)
    if _trace:
        return loss, res
    return loss
